# revision 3
# baseline (speedup 1.0000x reference)
"""Trainium2 Bass kernel for nn_Attention_8839042695176 (factored softmax).

Full inputs in, full output out. Core h owns attention head h (both batches).

Math per (b, h) unit, exploiting exp-separability of the positional logits
and the 64x-block structure of the upsampled conv-attention dots:
    N[i,j] = exp(ph[:,jy]q_i + dots[a(i), j>>6]) * exp(pw[:,jx]q_i)
with j = 48*jy + jx, a(i) = i>>6. Each jy-row of j crosses at most one
64-boundary, so (jy, d) pairs pack into NS=72 columns s (48 seg1 + 24 seg2):
    AD[i,s]  = exp(ph[:,jy(s)]q_i + dots[a(i), d(s)])        [i, 72]
    U[i,c,s] = sum_jx exp(pw[:,jx]q_i) * vv[c,48jy(s)+jx] * mask[jx,s]
    out[c,i] = sum_s AD[i,s]U[i,c,s] / sum_s AD[i,s]U[i,8,s]  (c=8: ones)
PE does all contractions (U via 48->324-wide matmuls per i-tile); DVE/Pool
do the [128,9,72] product; DVE tensor_reduce does the s-sum. The big
[2304,2304] exp of the baseline disappears entirely.
"""

import numpy as np

HEADS = 8
DIM_HEAD = 8
B = 2
C = 64
H = 48
HW = H * H            # 2304
KS = 11
PAD = 2
M6 = 6
MM = 36
SCALE = DIM_HEAD ** (-0.5)
NCORES = 8
NT = HW // 128        # 18 i-tiles
NS = 72               # packed (seg, jy) columns
GC1 = 0.7978845608028654          # sqrt(2/pi)
GC2 = GC1 * 0.044715

# s-index structures
_D1 = [(48 * jy) >> 6 for jy in range(48)]
_CROSS = [jy for jy in range(48) if ((48 * jy + 47) >> 6) != _D1[jy]]  # 24
JY_OF_S = list(range(48)) + _CROSS
D_OF_S = _D1 + [_D1[jy] + 1 for jy in _CROSS]

CHUNKS = [(0, 512), (512, 512), (1024, 512), (1536, 512), (2048, 256)]

_PROGRAMS = {}
# Per-tile engine assignment for the [128,9,72] product.
# MULT_MODE: 0 = DVE mult straight from PSUM (1x), 1 = ACT copy to SBUF bf16
# then DVE mult (2x mode), 2 = ACT copy then Pool mult (Pool can't read PSUM).
_MPAT = {0: 1, 1: 2, 2: 2}
MULT_MODE = [_MPAT[i % 3] for i in range(B * NT)]


def _build_program(repeat=1, split=True):
    from contextlib import ExitStack
    import concourse.bass as bass
    import concourse.mybir as mybir
    import concourse.tile as tile

    F32 = mybir.dt.float32
    BF = mybir.dt.bfloat16
    AF = mybir.ActivationFunctionType
    ALU = mybir.AluOpType

    nc = bass.Bass(trn_type="TRN2")

    f2 = nc.declare_dram_parameter("f2", [C, B * HW], BF, isOutput=False)
    w1T = nc.declare_dram_parameter("w1T", [C, 128], BF, isOutput=False)
    wvT = nc.declare_dram_parameter("wvT", [C, 8], BF, isOutput=False)
    wc2 = nc.declare_dram_parameter("wc2", [128, 121 * 16], BF, isOutput=False)
    bqk = nc.declare_dram_parameter("bqk", [16], F32, isOutput=False)
    phD = nc.declare_dram_parameter("phD", [8, NS], BF, isOutput=False)
    pw8 = nc.declare_dram_parameter("pw8", [8, H], BF, isOutput=False)
    e36 = nc.declare_dram_parameter("e36", [36, HW], BF, isOutput=False)
    sel36 = nc.declare_dram_parameter("sel36", [36, NS], BF, isOutput=False)
    mask48 = nc.declare_dram_parameter("mask48", [H, NS], BF, isOutput=False)
    outT = nc.declare_dram_parameter("outT", [B, HW, 8], F32, isOutput=True)

    def sap(t, off, dims):
        return bass.AP(tensor=t.tensor, offset=t.offset + off,
                       ap=[[t.ap[0][0], dims[0][1]] if dims[0][0] is None else dims[0]]
                       + list(dims[1:]))

    with tile.TileContext(nc) as tc, ExitStack() as ctx:
        const = ctx.enter_context(tc.tile_pool(name="const", bufs=1))
        work = ctx.enter_context(tc.tile_pool(name="work", bufs=3))

        def _rep_body():
            # ---- persistent tiles ----
            F = const.tile([C, B, HW], BF)
            W1 = const.tile([C, 128], BF)
            WC = const.tile([128, 121 * 16], BF)
            WV = const.tile([C, 8], BF)
            BQK = const.tile([16, 1], F32)
            PW = const.tile([8, H], BF)
            SEL = const.tile([36, NS], BF)
            MASK = const.tile([H, NS], BF)
            Q2 = const.tile([128, B, HW], BF)
            PHD = const.tile([8, NS], BF)
            E36 = const.tile([36, HW], BF)
            BxT = const.tile([H, B, HW], BF)
            ADT = const.tile([128, B, NT, NS], BF)
            MP = const.tile([H, B, 9, NS], BF)
            QKD = const.tile([16, B, MM], BF)
            KD8 = const.tile([8, B, MM], BF)
            DLSb = const.tile([36, B, NS], BF)
            OUTT = const.tile([128, B, NT, 9], F32)

            nc.sync.dma_start(F, f2[:, :].rearrange("p (b j) -> p b j", b=B))
            nc.sync.dma_start(W1, w1T[:, :])
            nc.sync.dma_start(WC, wc2[:, :])
            nc.sync.dma_start(WV, wvT[:, :])
            nc.sync.dma_start(BQK, bqk[:].rearrange("(p f) -> p f", f=1))
            nc.sync.dma_start(PW, pw8[:, :])
            nc.sync.dma_start(SEL, sel36[:, :])
            nc.sync.dma_start(MASK, mask48[:, :])
            nc.sync.dma_start(PHD, phD[:, :])
            nc.sync.dma_start(E36, e36[:, :])

            pM = MP.ap[0][0]
            p128 = Q2.ap[0][0]

            with tc.tile_pool(name="pro", bufs=1) as pro, \
                 tc.tile_pool(name="ppro", bufs=1, space="PSUM") as pp:
                # ---- qkv projection + staging ----
                for b in range(B):
                    for (j0, nj) in CHUNKS:
                        pq = pp.tile([128, 512], F32, tag="pq", bufs=2)
                        nc.tensor.matmul(pq[:, :nj], lhsT=W1,
                                         rhs=F[:, b, j0:j0 + nj],
                                         start=True, stop=True)
                        # split staging across ACT/DVE so conv starts sooner
                        if b == 0:
                            nc.scalar.activation(Q2[:, b, j0:j0 + nj],
                                                 pq[:, :nj], AF.Copy)
                        else:
                            nc.vector.tensor_copy(Q2[:, b, j0:j0 + nj],
                                                  pq[:, :nj])

                # ---- Mpack: vv slabs via per-jy matmuls, then mask ----
                for b in range(B):
                    MPP = pp.tile([H, 512], F32, tag="mpp", bufs=1)
                    for jy in range(48):
                        dst = bass.AP(tensor=MPP.tensor,
                                      offset=MPP.offset + jy * 8,
                                      ap=[[MPP.ap[0][0], H], [1, 8]])
                        nc.tensor.matmul(dst, lhsT=F[:, b, 48 * jy:48 * jy + 48],
                                         rhs=WV, start=True, stop=True)
                    mpp = MPP.ap[0][0]
                    pmask = MASK.ap[0][0]
                    # seg1 columns (s = jy)
                    nc.vector.tensor_mul(
                        bass.AP(tensor=MP.tensor, offset=MP.offset + b * 9 * NS,
                                ap=[[pM, H], [NS, 8], [1, 48]]),
                        bass.AP(tensor=MPP.tensor, offset=MPP.offset,
                                ap=[[mpp, H], [1, 8], [8, 48]]),
                        bass.AP(tensor=MASK.tensor, offset=MASK.offset,
                                ap=[[pmask, H], [0, 8], [1, 48]]))
                    # seg2 columns gather the crossing jy = 4g + r, r in {1,2}
                    nc.vector.tensor_mul(
                        bass.AP(tensor=MP.tensor,
                                offset=MP.offset + b * 9 * NS + 48,
                                ap=[[pM, H], [NS, 8], [2, 12], [1, 2]]),
                        bass.AP(tensor=MPP.tensor, offset=MPP.offset + 8,
                                ap=[[mpp, H], [1, 8], [32, 12], [8, 2]]),
                        bass.AP(tensor=MASK.tensor, offset=MASK.offset + 48,
                                ap=[[pmask, H], [0, 8], [2, 12], [1, 2]]))
                    nc.vector.tensor_copy(MP[:, b, 8, :], MASK[:, :])

                # ---- strided 11x11 convs, q+k stacked in K, b in free ----
                ACC = pp.tile([16, B, MM], F32, tag="acc", bufs=1)
                taps = [(2, 2)] + [(ky, kx) for ky in range(KS) for kx in range(KS)
                                   if (ky, kx) != (2, 2)]
                for ti, (ky, kx) in enumerate(taps):
                    oy0 = 1 if ky < 2 else 0
                    noy = 5 if (ky < 2 or ky == 10) else 6
                    ox0 = 1 if kx < 2 else 0
                    nox = 5 if (kx < 2 or kx == 10) else 6
                    rhs = bass.AP(
                        tensor=Q2.tensor,
                        offset=Q2.offset + (8 * oy0 + ky - PAD) * H
                        + (8 * ox0 + kx - PAD),
                        ap=[[p128, 128], [HW, B], [8 * H, noy], [8, nox]])
                    out = bass.AP(
                        tensor=ACC.tensor,
                        offset=ACC.offset + 6 * oy0 + ox0,
                        ap=[[ACC.ap[0][0], 16], [MM, B], [6, noy], [1, nox]])
                    tau = ky * KS + kx
                    nc.tensor.matmul(out, lhsT=WC[:, tau * 16:(tau + 1) * 16],
                                     rhs=rhs, start=(ti == 0),
                                     stop=(ti == len(taps) - 1))
                # gelu (tanh approx; the 0.5 folds into the dots scale)
                X = pro.tile([16, B * MM], F32, name="X")
                SQ = pro.tile([16, B * MM], F32, name="SQ")
                T1 = pro.tile([16, B * MM], F32, name="T1")
                T2 = pro.tile([16, B * MM], F32, name="T2")
                T3 = pro.tile([16, B * MM], F32, name="T3")
                accf = ACC.rearrange("p b m -> p (b m)")
                nc.scalar.activation(X, accf, AF.Identity, bias=BQK)
                nc.scalar.activation(SQ, accf, AF.Square, bias=BQK)
                nc.vector.tensor_scalar(T1, SQ, GC2, GC1, ALU.mult, ALU.add)
                nc.vector.tensor_mul(T2, T1, X)
                nc.scalar.activation(T3, T2, AF.Tanh)
                nc.vector.scalar_tensor_tensor(
                    QKD.rearrange("p b m -> p (b m)"), T3, 1.0, X,
                    ALU.add, ALU.mult)
                nc.sync.dma_start(KD8, QKD[8:16, :, :])

                # ---- dots^T -> DlogSel per b ----
                for b in range(B):
                    DT = pp.tile([36, NS], F32, tag="dt", bufs=1)
                    nc.tensor.matmul(DT[:, 0:36], lhsT=KD8[:, b, :],
                                     rhs=QKD[0:8, b, :], start=True, stop=True)
                    DTS = pro.tile([36, 36], BF, tag="dts", name="DTS")
                    nc.scalar.activation(DTS, DT[:, 0:36], AF.Copy,
                                         scale=SCALE * 0.25)
                    DLS = pp.tile([36, NS], F32, tag="dt", bufs=1)
                    nc.tensor.matmul(DLS, lhsT=DTS, rhs=SEL,
                                     start=True, stop=True)
                    nc.scalar.activation(DLSb[:, b, :], DLS, AF.Copy)

                # ---- Bx = exp(pw^T q8) ----
                for b in range(B):
                    for (i0, ni) in CHUNKS:
                        XS = pp.tile([H, 512], F32, tag="xs", bufs=2)
                        nc.tensor.matmul(XS[:, :ni], lhsT=PW,
                                         rhs=Q2[0:8, b, i0:i0 + ni],
                                         start=True, stop=True)
                        nc.scalar.activation(BxT[:, b, i0:i0 + ni], XS[:, :ni],
                                             AF.Exp)

            # ---- main loops ----
            with tc.tile_pool(name="pmain", bufs=1, space="PSUM") as pm:
                for b in range(B):
                    # AD = exp(ph q + dots expand), 3 i-tiles per exp
                    for g in range(NT // 3):
                        ADS = pm.tile([128, 216], F32, tag="ads", bufs=2)
                        for u in range(3):
                            t = 3 * g + u
                            nc.tensor.matmul(
                                ADS[:, 72 * u:72 * u + 72],
                                lhsT=Q2[0:8, b, 128 * t:128 * (t + 1)],
                                rhs=PHD, start=True, stop=False)
                            nc.tensor.matmul(
                                ADS[:, 72 * u:72 * u + 72],
                                lhsT=E36[:, 128 * t:128 * (t + 1)],
                                rhs=DLSb[:, b, :], start=False, stop=True)
                        nc.scalar.activation(
                            ADT[:, b, 3 * g:3 * g + 3, :],
                            ADS.rearrange("p (a c) -> p a c", a=3, c=NS),
                            AF.Exp)
                    for t in range(NT):
                        UT = pm.tile([128, 2, 512], F32, tag="ut", bufs=3)
                        for bank in range(2):
                            dst = bass.AP(tensor=UT.tensor,
                                          offset=UT.offset + bank * 512,
                                          ap=[[UT.ap[0][0], 128], [1, 324]])
                            rhs = bass.AP(tensor=MP.tensor,
                                          offset=MP.offset + b * 9 * NS + 36 * bank,
                                          ap=[[pM, H], [NS, 9], [1, 36]])
                            nc.tensor.matmul(dst,
                                             lhsT=BxT[:, b, 128 * t:128 * (t + 1)],
                                             rhs=rhs, start=True, stop=True)
                        idx = b * NT + t
                        mode = MULT_MODE[idx]
                        P = work.tile([128, 9 * NS], BF, tag="prod", name="P")
                        pout = bass.AP(tensor=P.tensor, offset=P.offset,
                                       ap=[[P.ap[0][0], 128], [NS, 9], [36, 2], [1, 36]])
                        pin0 = bass.AP(tensor=ADT.tensor,
                                       offset=ADT.offset + (b * NT + t) * NS,
                                       ap=[[ADT.ap[0][0], 128], [0, 9], [36, 2], [1, 36]])
                        if mode == 0:
                            pin1 = bass.AP(tensor=UT.tensor, offset=UT.offset,
                                           ap=[[UT.ap[0][0], 128], [36, 9], [512, 2], [1, 36]])
                            nc.vector.tensor_mul(pout, pin0, pin1)
                        else:
                            US = work.tile([128, 2, 324], BF, tag="us", name="US")
                            cin = bass.AP(tensor=UT.tensor, offset=UT.offset,
                                          ap=[[UT.ap[0][0], 128], [512, 2], [1, 324]])
                            nc.scalar.activation(US, cin, AF.Copy)
                            pin1 = bass.AP(tensor=US.tensor, offset=US.offset,
                                           ap=[[US.ap[0][0], 128], [36, 9], [324, 2], [1, 36]])
                            eng = nc.gpsimd if mode == 2 else nc.vector
                            eng.tensor_mul(pout, pin0, pin1)
                        nc.vector.tensor_reduce(
                            OUTT[:, b, t, :],
                            P.rearrange("p (c s) -> p c s", c=9, s=NS),
                            mybir.AxisListType.X, ALU.add)
                    # final: divide and store (i-major; host transposes back)
                    REC = work.tile([128, NT], F32, tag="rec", name="REC")
                    den = bass.AP(tensor=OUTT.tensor,
                                  offset=OUTT.offset + b * NT * 9 + 8,
                                  ap=[[OUTT.ap[0][0], 128], [9, NT]])
                    nc.vector.reciprocal(REC, den)
                    RES = work.tile([128, NT, 8], F32, tag="res", name="RES")
                    num = bass.AP(tensor=OUTT.tensor,
                                  offset=OUTT.offset + b * NT * 9,
                                  ap=[[OUTT.ap[0][0], 128], [9, NT], [1, 8]])
                    rb = bass.AP(tensor=REC.tensor, offset=REC.offset,
                                 ap=[[REC.ap[0][0], 128], [1, NT], [0, 8]])
                    nc.vector.tensor_mul(RES, num, rb)
                    nc.sync.dma_start(
                        outT[b, :, :].rearrange("(t p) c -> p t c", t=NT, p=128),
                        RES)

        for _rep in range(repeat):
            _rep_body()

    if split:
        _split_waits(nc)
    return nc


def _split_waits(nc):
    """Walrus allows at most ONE sync-wait per instruction; move extras onto
    same-engine NoOps."""
    import concourse.mybir as mybir
    ctr = 0
    for fn in nc.m.functions:
        for blk in fn.blocks:
            new = []
            for inst in blk.instructions:
                si = inst.sync_info
                waits = list(si.on_wait) if si and si.on_wait else []
                if len(waits) > 1:
                    for w in waits[:-1]:
                        ctr += 1
                        nop = mybir.InstNoOp(name=f"I-wsplit-{ctr}", ins=[], outs=[])
                        nop.engine = inst.engine
                        nop.sync_info = mybir.SyncInfo(on_wait=[w], on_update=[])
                        new.append(nop)
                    inst.sync_info = mybir.SyncInfo(
                        on_wait=[waits[-1]],
                        on_update=list(si.on_update or []))
                new.append(inst)
            blk.instructions = new


def _get_program(repeat=1):
    if repeat not in _PROGRAMS:
        _PROGRAMS[repeat] = _build_program(repeat)
    return _PROGRAMS[repeat]


def _make_in_maps(f, w_qkv, wq, bq, wk, bk, pos_h, pos_w):
    import ml_dtypes
    BFD = ml_dtypes.bfloat16
    f = np.asarray(f, np.float32)
    w = np.asarray(w_qkv, np.float32)[:, :, 0, 0]
    wq = np.asarray(wq, np.float32)
    wk = np.asarray(wk, np.float32)
    bq = np.asarray(bq, np.float32)
    bk = np.asarray(bk, np.float32)
    pos_h = np.asarray(pos_h, np.float32)
    pos_w = np.asarray(pos_w, np.float32)

    f2 = np.ascontiguousarray(
        f.reshape(B, C, HW).transpose(1, 0, 2).reshape(C, B * HW)).astype(BFD)
    e36 = np.zeros((36, HW), np.float32)
    e36[np.arange(HW) >> 6, np.arange(HW)] = 1.0
    sel36 = np.zeros((36, NS), np.float32)
    sel36[D_OF_S, np.arange(NS)] = 1.0
    mask48 = np.zeros((H, NS), np.float32)
    for s in range(NS):
        j = 48 * JY_OF_S[s] + np.arange(48)
        mask48[:, s] = ((j >> 6) == D_OF_S[s]).astype(np.float32)
    phD = pos_h[:, JY_OF_S]

    in_maps = []
    for h in range(NCORES):
        head = np.arange(h * 8, h * 8 + 8)
        rest = np.delete(np.arange(C), head)
        perm = np.concatenate([head, rest])
        w1T = np.ascontiguousarray(
            np.concatenate([w[0:C][perm].T, w[C:2 * C].T], axis=1)).astype(BFD)
        wvT = np.ascontiguousarray(w[2 * C + 8 * h:2 * C + 8 * h + 8].T).astype(BFD)
        wqh = wq[head][:, perm]     # [8, 64, 11, 11], in-ch in q-map order
        wkh = wk[head]              # k-map in natural order
        wc2 = np.zeros((128, 121 * 16), np.float32)
        for ky in range(KS):
            for kx in range(KS):
                tau = ky * KS + kx
                wc2[0:64, tau * 16:tau * 16 + 8] = wqh[:, :, ky, kx].T
                wc2[64:128, tau * 16 + 8:tau * 16 + 16] = wkh[:, :, ky, kx].T
        in_maps.append({
            "f2": f2,
            "w1T": w1T,
            "wvT": wvT,
            "wc2": np.ascontiguousarray(wc2).astype(BFD),
            "bqk": np.ascontiguousarray(
                np.concatenate([bq[head], bk[head]])).astype(np.float32),
            "phD": np.ascontiguousarray(phD).astype(BFD),
            "pw8": np.ascontiguousarray(pos_w).astype(BFD),
            "e36": np.ascontiguousarray(e36).astype(BFD),
            "sel36": np.ascontiguousarray(sel36).astype(BFD),
            "mask48": np.ascontiguousarray(mask48).astype(BFD),
        })
    return in_maps


def _assemble(results):
    fmap = np.empty((B, C, HW), np.float32)
    for h in range(NCORES):
        fmap[:, h * 8:(h + 1) * 8, :] = results[h]["outT"].transpose(0, 2, 1)
    return fmap.reshape(B, C, H, H)


def run(trace=False, **inputs):
    from concourse.bass_utils import run_bass_kernel_spmd
    nc = _get_program()
    in_maps = _make_in_maps(**inputs)
    res = run_bass_kernel_spmd(nc, in_maps, core_ids=list(range(NCORES)),
                               trace=trace)
    return _assemble(res.results), res


def kernel(**inputs):
    out, _ = run(trace=False, **inputs)
    return out


# revision 4
# speedup vs baseline: 2.4273x; 2.4273x over previous
"""Trainium2 Bass kernel for nn_Attention_8839042695176 (factored softmax).

Full inputs in, full output out. Core h owns attention head h (both batches).

Math per (b, h) unit, exploiting exp-separability of the positional logits
and the 64x-block structure of the upsampled conv-attention dots:
    N[i,j] = exp(ph[:,jy]q_i + dots[a(i), j>>6]) * exp(pw[:,jx]q_i)
with j = 48*jy + jx, a(i) = i>>6. Each jy-row of j crosses at most one
64-boundary, so (jy, d) pairs pack into NS=72 columns s (48 seg1 + 24 seg2):
    AD[i,s]  = exp(ph[:,jy(s)]q_i + dots[a(i), d(s)])        [i, 72]
    U[i,c,s] = sum_jx exp(pw[:,jx]q_i) * vv[c,48jy(s)+jx] * mask[jx,s]
    out[c,i] = sum_s AD[i,s]U[i,c,s] / sum_s AD[i,s]U[i,8,s]  (c=8: ones)
PE does all contractions (U via 48->324-wide matmuls per i-tile); DVE/Pool
do the [128,9,72] product; DVE tensor_reduce does the s-sum. The big
[2304,2304] exp of the baseline disappears entirely.
"""

import numpy as np

HEADS = 8
DIM_HEAD = 8
B = 2
C = 64
H = 48
HW = H * H            # 2304
KS = 11
PAD = 2
M6 = 6
MM = 36
SCALE = DIM_HEAD ** (-0.5)
NCORES = 8
NT = HW // 128        # 18 i-tiles
NS = 72               # packed (seg, jy) columns
GC1 = 0.7978845608028654          # sqrt(2/pi)
GC2 = GC1 * 0.044715

# s-index structures
_D1 = [(48 * jy) >> 6 for jy in range(48)]
_CROSS = [jy for jy in range(48) if ((48 * jy + 47) >> 6) != _D1[jy]]  # 24
JY_OF_S = list(range(48)) + _CROSS
D_OF_S = _D1 + [_D1[jy] + 1 for jy in _CROSS]

CHUNKS = [(0, 512), (512, 512), (1024, 512), (1536, 512), (2048, 256)]

_PROGRAMS = {}
# Per-tile engine assignment for the [128,9,72] product.
# MULT_MODE: 0 = DVE mult straight from PSUM (1x), 1 = ACT copy to SBUF bf16
# then DVE mult (2x mode), 2 = ACT copy then Pool mult (Pool can't read PSUM).
_MPAT = {0: 1, 1: 2, 2: 2}
MULT_MODE = [_MPAT[i % 3] for i in range(B * NT)]


def _build_program(repeat=1, split=True):
    from contextlib import ExitStack
    import concourse.bass as bass
    import concourse.mybir as mybir
    import concourse.tile as tile

    F32 = mybir.dt.float32
    BF = mybir.dt.bfloat16
    AF = mybir.ActivationFunctionType
    ALU = mybir.AluOpType

    nc = bass.Bass(trn_type="TRN2")

    f2 = nc.declare_dram_parameter("f2", [C, B * HW], BF, isOutput=False)
    w1T = nc.declare_dram_parameter("w1T", [C, 128], BF, isOutput=False)
    wvT = nc.declare_dram_parameter("wvT", [C, 8], BF, isOutput=False)
    wc2 = nc.declare_dram_parameter("wc2", [128, 121 * 16], BF, isOutput=False)
    bqk = nc.declare_dram_parameter("bqk", [16], F32, isOutput=False)
    phD = nc.declare_dram_parameter("phD", [8, NS], BF, isOutput=False)
    pw8 = nc.declare_dram_parameter("pw8", [8, H], BF, isOutput=False)
    e36 = nc.declare_dram_parameter("e36", [36, HW], BF, isOutput=False)
    sel36 = nc.declare_dram_parameter("sel36", [36, NS], BF, isOutput=False)
    mask48 = nc.declare_dram_parameter("mask48", [H, NS], BF, isOutput=False)
    outT = nc.declare_dram_parameter("outT", [B, HW, 8], F32, isOutput=True)

    def sap(t, off, dims):
        return bass.AP(tensor=t.tensor, offset=t.offset + off,
                       ap=[[t.ap[0][0], dims[0][1]] if dims[0][0] is None else dims[0]]
                       + list(dims[1:]))

    with tile.TileContext(nc) as tc, ExitStack() as ctx:
        const = ctx.enter_context(tc.tile_pool(name="const", bufs=1))
        work = ctx.enter_context(tc.tile_pool(name="work", bufs=3))

        def _rep_body():
            # ---- persistent tiles ----
            F = const.tile([C, B, HW], BF)
            W1 = const.tile([C, 128], BF)
            WC = const.tile([128, 121 * 16], BF)
            WV = const.tile([C, 8], BF)
            BQK = const.tile([16, 1], F32)
            PW = const.tile([8, H], BF)
            SEL = const.tile([36, NS], BF)
            MASK = const.tile([H, NS], BF)
            Q2 = const.tile([128, B, HW], BF)
            PHD = const.tile([8, NS], BF)
            E36 = const.tile([36, HW], BF)
            BxT = const.tile([H, B, HW], BF)
            ADT = const.tile([128, B, NT, NS], BF)
            MP = const.tile([H, B, 9, NS], BF)
            QKD = const.tile([16, B, MM], BF)
            KD8 = const.tile([8, B, MM], BF)
            DLSb = const.tile([36, B, NS], BF)
            OUTT = const.tile([128, B, NT, 9], F32)

            nc.sync.dma_start(F, f2[:, :].rearrange("p (b j) -> p b j", b=B))
            nc.sync.dma_start(W1, w1T[:, :])
            nc.sync.dma_start(WC, wc2[:, :])
            nc.sync.dma_start(WV, wvT[:, :])
            nc.sync.dma_start(BQK, bqk[:].rearrange("(p f) -> p f", f=1))
            nc.sync.dma_start(PW, pw8[:, :])
            nc.sync.dma_start(SEL, sel36[:, :])
            nc.sync.dma_start(MASK, mask48[:, :])
            nc.sync.dma_start(PHD, phD[:, :])
            nc.sync.dma_start(E36, e36[:, :])

            pM = MP.ap[0][0]
            p128 = Q2.ap[0][0]

            with tc.tile_pool(name="pro", bufs=1) as pro, \
                 tc.tile_pool(name="ppro", bufs=1, space="PSUM") as pp:
                # ---- qkv projection + staging ----
                for b in range(B):
                    for (j0, nj) in CHUNKS:
                        pq = pp.tile([128, 512], F32, tag="pq", bufs=2)
                        nc.tensor.matmul(pq[:, :nj], lhsT=W1,
                                         rhs=F[:, b, j0:j0 + nj],
                                         start=True, stop=True)
                        # split staging across ACT/DVE so conv starts sooner
                        if b == 0:
                            nc.scalar.activation(Q2[:, b, j0:j0 + nj],
                                                 pq[:, :nj], AF.Copy)
                        else:
                            nc.vector.tensor_copy(Q2[:, b, j0:j0 + nj],
                                                  pq[:, :nj])

                # ---- strided 11x11 convs, q+k stacked in K, b in free ----
                ACC = pp.tile([16, B, MM], F32, tag="acc", bufs=1)
                taps = [(2, 2)] + [(ky, kx) for ky in range(KS) for kx in range(KS)
                                   if (ky, kx) != (2, 2)]
                for ti, (ky, kx) in enumerate(taps):
                    oy0 = 1 if ky < 2 else 0
                    noy = 5 if (ky < 2 or ky == 10) else 6
                    ox0 = 1 if kx < 2 else 0
                    nox = 5 if (kx < 2 or kx == 10) else 6
                    rhs = bass.AP(
                        tensor=Q2.tensor,
                        offset=Q2.offset + (8 * oy0 + ky - PAD) * H
                        + (8 * ox0 + kx - PAD),
                        ap=[[p128, 128], [HW, B], [8 * H, noy], [8, nox]])
                    out = bass.AP(
                        tensor=ACC.tensor,
                        offset=ACC.offset + 6 * oy0 + ox0,
                        ap=[[ACC.ap[0][0], 16], [MM, B], [6, noy], [1, nox]])
                    tau = ky * KS + kx
                    nc.tensor.matmul(out, lhsT=WC[:, tau * 16:(tau + 1) * 16],
                                     rhs=rhs, start=(ti == 0),
                                     stop=(ti == len(taps) - 1))
                # gelu (tanh approx; the 0.5 folds into the dots scale)
                X = pro.tile([16, B * MM], F32, name="X")
                SQ = pro.tile([16, B * MM], F32, name="SQ")
                T1 = pro.tile([16, B * MM], F32, name="T1")
                T2 = pro.tile([16, B * MM], F32, name="T2")
                T3 = pro.tile([16, B * MM], F32, name="T3")
                accf = ACC.rearrange("p b m -> p (b m)")
                nc.scalar.activation(X, accf, AF.Identity, bias=BQK)
                nc.scalar.activation(SQ, accf, AF.Square, bias=BQK)
                nc.vector.tensor_scalar(T1, SQ, GC2, GC1, ALU.mult, ALU.add)
                nc.vector.tensor_mul(T2, T1, X)
                nc.scalar.activation(T3, T2, AF.Tanh)
                nc.vector.scalar_tensor_tensor(
                    QKD.rearrange("p b m -> p (b m)"), T3, 1.0, X,
                    ALU.add, ALU.mult)
                nc.sync.dma_start(KD8, QKD[8:16, :, :])

                # ---- dots^T -> DlogSel per b ----
                for b in range(B):
                    DT = pp.tile([36, NS], F32, tag="dt", bufs=1)
                    nc.tensor.matmul(DT[:, 0:36], lhsT=KD8[:, b, :],
                                     rhs=QKD[0:8, b, :], start=True, stop=True)
                    DTS = pro.tile([36, 36], BF, tag="dts", name="DTS")
                    nc.scalar.activation(DTS, DT[:, 0:36], AF.Copy,
                                         scale=SCALE * 0.25)
                    DLS = pp.tile([36, NS], F32, tag="dt", bufs=1)
                    nc.tensor.matmul(DLS, lhsT=DTS, rhs=SEL,
                                     start=True, stop=True)
                    nc.scalar.activation(DLSb[:, b, :], DLS, AF.Copy)

                # ---- Bx = exp(pw^T q8) ----
                for b in range(B):
                    for (i0, ni) in CHUNKS:
                        XS = pp.tile([H, 512], F32, tag="xs", bufs=2)
                        nc.tensor.matmul(XS[:, :ni], lhsT=PW,
                                         rhs=Q2[0:8, b, i0:i0 + ni],
                                         start=True, stop=True)
                        nc.scalar.activation(BxT[:, b, i0:i0 + ni], XS[:, :ni],
                                             AF.Exp)

                # ---- Mpack (after conv: only needed by U-matmuls) ----
                # ---- Mpack: vv slabs via per-jy matmuls, then mask ----
                for b in range(B):
                    MPP = pp.tile([H, 512], F32, tag="mpp", bufs=1)
                    for jy in range(48):
                        dst = bass.AP(tensor=MPP.tensor,
                                      offset=MPP.offset + jy * 8,
                                      ap=[[MPP.ap[0][0], H], [1, 8]])
                        nc.tensor.matmul(dst, lhsT=F[:, b, 48 * jy:48 * jy + 48],
                                         rhs=WV, start=True, stop=True)
                    mpp = MPP.ap[0][0]
                    pmask = MASK.ap[0][0]
                    # seg1 columns (s = jy)
                    nc.vector.tensor_mul(
                        bass.AP(tensor=MP.tensor, offset=MP.offset + b * 9 * NS,
                                ap=[[pM, H], [NS, 8], [1, 48]]),
                        bass.AP(tensor=MPP.tensor, offset=MPP.offset,
                                ap=[[mpp, H], [1, 8], [8, 48]]),
                        bass.AP(tensor=MASK.tensor, offset=MASK.offset,
                                ap=[[pmask, H], [0, 8], [1, 48]]))
                    # seg2 columns gather the crossing jy = 4g + r, r in {1,2}
                    nc.vector.tensor_mul(
                        bass.AP(tensor=MP.tensor,
                                offset=MP.offset + b * 9 * NS + 48,
                                ap=[[pM, H], [NS, 8], [2, 12], [1, 2]]),
                        bass.AP(tensor=MPP.tensor, offset=MPP.offset + 8,
                                ap=[[mpp, H], [1, 8], [32, 12], [8, 2]]),
                        bass.AP(tensor=MASK.tensor, offset=MASK.offset + 48,
                                ap=[[pmask, H], [0, 8], [2, 12], [1, 2]]))
                    nc.vector.tensor_copy(MP[:, b, 8, :], MASK[:, :])

            # ---- main loops ----
            with tc.tile_pool(name="pmain", bufs=1, space="PSUM") as pm:
                for b in range(B):
                    # AD = exp(ph q + dots expand), 3 i-tiles per exp
                    for g in range(NT // 3):
                        ADS = pm.tile([128, 216], F32, tag="ads", bufs=2)
                        for u in range(3):
                            t = 3 * g + u
                            nc.tensor.matmul(
                                ADS[:, 72 * u:72 * u + 72],
                                lhsT=Q2[0:8, b, 128 * t:128 * (t + 1)],
                                rhs=PHD, start=True, stop=False)
                            nc.tensor.matmul(
                                ADS[:, 72 * u:72 * u + 72],
                                lhsT=E36[:, 128 * t:128 * (t + 1)],
                                rhs=DLSb[:, b, :], start=False, stop=True)
                        nc.scalar.activation(
                            ADT[:, b, 3 * g:3 * g + 3, :],
                            ADS.rearrange("p (a c) -> p a c", a=3, c=NS),
                            AF.Exp)
                    for t in range(NT):
                        UT = pm.tile([128, 2, 512], F32, tag="ut", bufs=3)
                        for bank in range(2):
                            dst = bass.AP(tensor=UT.tensor,
                                          offset=UT.offset + bank * 512,
                                          ap=[[UT.ap[0][0], 128], [1, 324]])
                            rhs = bass.AP(tensor=MP.tensor,
                                          offset=MP.offset + b * 9 * NS + 36 * bank,
                                          ap=[[pM, H], [NS, 9], [1, 36]])
                            nc.tensor.matmul(dst,
                                             lhsT=BxT[:, b, 128 * t:128 * (t + 1)],
                                             rhs=rhs, start=True, stop=True)
                        idx = b * NT + t
                        mode = MULT_MODE[idx]
                        P = work.tile([128, 9 * NS], BF, tag="prod", name="P")
                        pout = bass.AP(tensor=P.tensor, offset=P.offset,
                                       ap=[[P.ap[0][0], 128], [NS, 9], [36, 2], [1, 36]])
                        pin0 = bass.AP(tensor=ADT.tensor,
                                       offset=ADT.offset + (b * NT + t) * NS,
                                       ap=[[ADT.ap[0][0], 128], [0, 9], [36, 2], [1, 36]])
                        if mode == 0:
                            pin1 = bass.AP(tensor=UT.tensor, offset=UT.offset,
                                           ap=[[UT.ap[0][0], 128], [36, 9], [512, 2], [1, 36]])
                            nc.vector.tensor_mul(pout, pin0, pin1)
                        else:
                            US = work.tile([128, 2, 324], BF, tag="us", name="US")
                            cin = bass.AP(tensor=UT.tensor, offset=UT.offset,
                                          ap=[[UT.ap[0][0], 128], [512, 2], [1, 324]])
                            nc.scalar.activation(US, cin, AF.Copy)
                            pin1 = bass.AP(tensor=US.tensor, offset=US.offset,
                                           ap=[[US.ap[0][0], 128], [36, 9], [324, 2], [1, 36]])
                            eng = nc.gpsimd if mode == 2 else nc.vector
                            eng.tensor_mul(pout, pin0, pin1)
                        nc.vector.tensor_reduce(
                            OUTT[:, b, t, :],
                            P.rearrange("p (c s) -> p c s", c=9, s=NS),
                            mybir.AxisListType.X, ALU.add)
                    # final: divide and store (i-major; host transposes back)
                    REC = work.tile([128, NT], F32, tag="rec", name="REC")
                    den = bass.AP(tensor=OUTT.tensor,
                                  offset=OUTT.offset + b * NT * 9 + 8,
                                  ap=[[OUTT.ap[0][0], 128], [9, NT]])
                    nc.vector.reciprocal(REC, den)
                    RES = work.tile([128, NT, 8], F32, tag="res", name="RES")
                    num = bass.AP(tensor=OUTT.tensor,
                                  offset=OUTT.offset + b * NT * 9,
                                  ap=[[OUTT.ap[0][0], 128], [9, NT], [1, 8]])
                    rb = bass.AP(tensor=REC.tensor, offset=REC.offset,
                                 ap=[[REC.ap[0][0], 128], [1, NT], [0, 8]])
                    nc.vector.tensor_mul(RES, num, rb)
                    nc.sync.dma_start(
                        outT[b, :, :].rearrange("(t p) c -> p t c", t=NT, p=128),
                        RES)

        for _rep in range(repeat):
            _rep_body()

    if split:
        _split_waits(nc)
    return nc


def _split_waits(nc):
    """Walrus allows at most ONE sync-wait per instruction; move extras onto
    same-engine NoOps."""
    import concourse.mybir as mybir
    ctr = 0
    for fn in nc.m.functions:
        for blk in fn.blocks:
            new = []
            for inst in blk.instructions:
                si = inst.sync_info
                waits = list(si.on_wait) if si and si.on_wait else []
                if len(waits) > 1:
                    for w in waits[:-1]:
                        ctr += 1
                        nop = mybir.InstNoOp(name=f"I-wsplit-{ctr}", ins=[], outs=[])
                        nop.engine = inst.engine
                        nop.sync_info = mybir.SyncInfo(on_wait=[w], on_update=[])
                        new.append(nop)
                    inst.sync_info = mybir.SyncInfo(
                        on_wait=[waits[-1]],
                        on_update=list(si.on_update or []))
                new.append(inst)
            blk.instructions = new


def _get_program(repeat=1):
    if repeat not in _PROGRAMS:
        _PROGRAMS[repeat] = _build_program(repeat)
    return _PROGRAMS[repeat]


def _make_in_maps(f, w_qkv, wq, bq, wk, bk, pos_h, pos_w):
    import ml_dtypes
    BFD = ml_dtypes.bfloat16
    f = np.asarray(f, np.float32)
    w = np.asarray(w_qkv, np.float32)[:, :, 0, 0]
    wq = np.asarray(wq, np.float32)
    wk = np.asarray(wk, np.float32)
    bq = np.asarray(bq, np.float32)
    bk = np.asarray(bk, np.float32)
    pos_h = np.asarray(pos_h, np.float32)
    pos_w = np.asarray(pos_w, np.float32)

    f2 = np.ascontiguousarray(
        f.reshape(B, C, HW).transpose(1, 0, 2).reshape(C, B * HW)).astype(BFD)
    e36 = np.zeros((36, HW), np.float32)
    e36[np.arange(HW) >> 6, np.arange(HW)] = 1.0
    sel36 = np.zeros((36, NS), np.float32)
    sel36[D_OF_S, np.arange(NS)] = 1.0
    mask48 = np.zeros((H, NS), np.float32)
    for s in range(NS):
        j = 48 * JY_OF_S[s] + np.arange(48)
        mask48[:, s] = ((j >> 6) == D_OF_S[s]).astype(np.float32)
    phD = pos_h[:, JY_OF_S]

    in_maps = []
    for h in range(NCORES):
        head = np.arange(h * 8, h * 8 + 8)
        rest = np.delete(np.arange(C), head)
        perm = np.concatenate([head, rest])
        w1T = np.ascontiguousarray(
            np.concatenate([w[0:C][perm].T, w[C:2 * C].T], axis=1)).astype(BFD)
        wvT = np.ascontiguousarray(w[2 * C + 8 * h:2 * C + 8 * h + 8].T).astype(BFD)
        wqh = wq[head][:, perm]     # [8, 64, 11, 11], in-ch in q-map order
        wkh = wk[head]              # k-map in natural order
        wc2 = np.zeros((128, 121 * 16), np.float32)
        for ky in range(KS):
            for kx in range(KS):
                tau = ky * KS + kx
                wc2[0:64, tau * 16:tau * 16 + 8] = wqh[:, :, ky, kx].T
                wc2[64:128, tau * 16 + 8:tau * 16 + 16] = wkh[:, :, ky, kx].T
        in_maps.append({
            "f2": f2,
            "w1T": w1T,
            "wvT": wvT,
            "wc2": np.ascontiguousarray(wc2).astype(BFD),
            "bqk": np.ascontiguousarray(
                np.concatenate([bq[head], bk[head]])).astype(np.float32),
            "phD": np.ascontiguousarray(phD).astype(BFD),
            "pw8": np.ascontiguousarray(pos_w).astype(BFD),
            "e36": np.ascontiguousarray(e36).astype(BFD),
            "sel36": np.ascontiguousarray(sel36).astype(BFD),
            "mask48": np.ascontiguousarray(mask48).astype(BFD),
        })
    return in_maps


def _assemble(results):
    fmap = np.empty((B, C, HW), np.float32)
    for h in range(NCORES):
        fmap[:, h * 8:(h + 1) * 8, :] = results[h]["outT"].transpose(0, 2, 1)
    return fmap.reshape(B, C, H, H)


def run(trace=False, **inputs):
    from concourse.bass_utils import run_bass_kernel_spmd
    nc = _get_program()
    in_maps = _make_in_maps(**inputs)
    res = run_bass_kernel_spmd(nc, in_maps, core_ids=list(range(NCORES)),
                               trace=trace)
    return _assemble(res.results), res


def kernel(**inputs):
    out, _ = run(trace=False, **inputs)
    return out
